# revision 1
# baseline (speedup 1.0000x reference)
"""DGCNN (4x EdgeConv + FC head) Bass kernel for 8 trn2 NeuronCores.

Sharding: cloud b -> cores {2b, 2b+1}; each core owns 1024 query points of its
cloud (q0 = (partition_id % 2) * 1024). Full cloud features are exchanged
within each pair via AllGather after layers 1-3.

Self-contained: hardcodes B=4, P=2048, K=20 and the model dims.
"""
import numpy as np

import concourse.bass as bass
import concourse.bacc as bacc
import concourse.mybir as mybir
import concourse.tile as tile
from concourse.bass_utils import run_bass_kernel_spmd
from concourse.masks import make_identity

B, P, K = 4, 2048, 20
NQ = 1024            # queries owned per core
N_CORES = 8
NEG = -3.0e38
LDIMS = [(3, 64, 64), (64, 128, 128), (128, 256, 256), (256, 512, 512)]
F32 = mybir.dt.float32
F32R = mybir.dt.float32r
AF = mybir.ActivationFunctionType
ALU = mybir.AluOpType
CCHUNK = 2           # neighbors per dma_gather (num_idxs = 128*CCHUNK <= 512)

_CACHED = {}


def cdiv(a, b):
    return (a + b - 1) // b


def _build():
    nc = bacc.Bacc("TRN2", target_bir_lowering=False, debug=False,
                   num_devices=N_CORES, num_swdge_queues=4)

    xT_in = nc.declare_dram_parameter("xT", [3, P], F32, isOutput=False)
    nsq_in = nc.declare_dram_parameter("nsq", [1, P], F32, isOutput=False)
    wparams = {}
    for li, (D, DH, DO) in enumerate(LDIMS, start=1):
        wparams[f"wdiff{li}"] = nc.declare_dram_parameter(f"wdiff{li}", [D, DH], F32, isOutput=False)
        wparams[f"wbot{li}"] = nc.declare_dram_parameter(f"wbot{li}", [D, DH], F32, isOutput=False)
        wparams[f"ba{li}"] = nc.declare_dram_parameter(f"ba{li}", [1, DH], F32, isOutput=False)
        wparams[f"wb{li}"] = nc.declare_dram_parameter(f"wb{li}", [DH, DO], F32, isOutput=False)
        wparams[f"bb{li}"] = nc.declare_dram_parameter(f"bb{li}", [DO, 1], F32, isOutput=False)
    wparams["fw1"] = nc.declare_dram_parameter("fw1", [960, 512], F32, isOutput=False)
    wparams["fb1"] = nc.declare_dram_parameter("fb1", [1, 512], F32, isOutput=False)
    wparams["fw2"] = nc.declare_dram_parameter("fw2", [512, 256], F32, isOutput=False)
    wparams["fb2"] = nc.declare_dram_parameter("fb2", [1, 256], F32, isOutput=False)
    wparams["fw3"] = nc.declare_dram_parameter("fw3", [256, 1], F32, isOutput=False)
    wparams["fb3"] = nc.declare_dram_parameter("fb3", [1, 1], F32, isOutput=False)
    y_out = nc.declare_dram_parameter("y", [1, NQ], F32, isOutput=True)

    groups = [[2 * b, 2 * b + 1] for b in range(N_CORES // 2)]

    with tile.TileContext(nc) as tc:
        with tc.tile_pool(name="const", bufs=1) as cpool, \
             tc.tile_pool(name="xping", bufs=1) as xping, \
             tc.tile_pool(name="xpong", bufs=1) as xpong, \
             tc.tile_pool(name="dram", bufs=1, space="DRAM") as dram:

            ident = cpool.tile([128, 128], F32)
            make_identity(nc, ident[:])
            onesr = cpool.tile([1, 1024], F32)
            nc.vector.memset(onesr[:], 1.0)
            onesr_r = cpool.tile([1, 1024], F32R)
            nc.vector.tensor_copy(onesr_r[:], onesr[:])
            onescol = cpool.tile([128, 1], F32)
            nc.vector.memset(onescol[:], 1.0)

            def load_round(pool, name, shape, row_chunks=None):
                """DRAM fp32 -> SBUF f32r tiles split at given row boundaries."""
                src = wparams[name]
                if row_chunks is None:
                    row_chunks = []
                    r = shape[0]
                    while r > 0:
                        row_chunks.append(min(128, r))
                        r -= 128
                tiles, c0 = [], 0
                for rows in row_chunks:
                    t32 = pool.tile([rows, shape[1]], F32, name=f"{name}_f{c0}",
                                    tag="wstage", bufs=2)
                    nc.sync.dma_start(t32[:], src[c0:c0 + rows, :])
                    tr = pool.tile([rows, shape[1]], F32R, name=f"{name}_r{c0}",
                                   tag=f"{name}_r{c0}")
                    nc.vector.tensor_copy(tr[:], t32[:])
                    tiles.append(tr)
                    c0 += rows
                return tiles

            q0 = nc.vector.partition_id()
            q0 = nc.vector.scalar_reg_alu(ALU.mod, q0, 2)
            q0 = nc.vector.scalar_reg_alu(ALU.mult, q0, NQ)

            nsq_dram = [nsq_in] + [dram.tile([1, P], F32, name=f"nsqd{li}")
                                   for li in (2, 3, 4)]
            xown_dram = [dram.tile([do, NQ], F32, name=f"xown{li}")
                         for li, (_, _, do) in enumerate(LDIMS, start=1)]
            ag_out = [dram.tile([2 * do, NQ], F32, name=f"agout{li}")
                      for li, (_, _, do) in enumerate(LDIMS[:3], start=1)]

            xT_tiles = None

            for li, (D, DH, DO) in enumerate(LDIMS, start=1):
                NDC = cdiv(D, 128)
                NHC = cdiv(DH, 128)
                NMC = cdiv(DO, 128)
                v_dram = dram.tile([P, DH], F32, name=f"vdram{li}")
                idx_dram = dram.tile([128, 8 * K], mybir.dt.int16, name=f"idxd{li}")

                with tc.tile_pool(name=f"l{li}", bufs=1) as lp, \
                     tc.tile_pool(name=f"l{li}w", bufs=2) as wp, \
                     tc.tile_pool(name=f"l{li}wt", bufs=1) as lw:

                    Wd = load_round(lw, f"wdiff{li}", (D, DH))
                    Wb = load_round(lw, f"wbot{li}", (D, DH))
                    Wba = load_round(lw, f"ba{li}", (1, DH))
                    Wwb = load_round(lw, f"wb{li}", (DH, DO))
                    Wbb = []
                    for m in range(NMC):
                        mrows = min(128, DO - m * 128)
                        bt = lw.tile([mrows, 1], F32, name=f"bb{li}_t{m}",
                                     tag=f"bb{li}_t{m}")
                        nc.sync.dma_start(bt[:], wparams[f"bb{li}"][m * 128:m * 128 + mrows, :])
                        Wbb.append(bt)

                    if li == 1:
                        t = lp.tile([3, P], F32, name="xT1")
                        nc.sync.dma_start(t[:], xT_in[:, :])
                        xT_tiles = [t]

                    xTr_tiles = []
                    for ci, xt in enumerate(xT_tiles):
                        tr = lp.tile([xt.shape[0], P], F32R, name=f"xTr{ci}")
                        nc.vector.tensor_copy(tr[:], xt[:])
                        xTr_tiles.append(tr)

                    QT, QTr = [], []
                    for ci, xt in enumerate(xT_tiles):
                        qt = lp.tile([xt.shape[0], NQ], F32, name=f"QT{ci}")
                        nc.vector.tensor_copy(qt[:], xt[:, bass.ds(q0, NQ)])
                        QT.append(qt)
                        qtr = lp.tile([xt.shape[0], NQ], F32R, name=f"QTr{ci}")
                        nc.vector.tensor_copy(qtr[:], xTr_tiles[ci][:, bass.ds(q0, NQ)])
                        QTr.append(qtr)

                    nsqb = lp.tile([128, P], F32, name="nsqb")
                    nc.sync.dma_start(nsqb[:],
                                      nsq_dram[li - 1][0:1, :].to_broadcast([128, P]))

                    # ---- phase 1: v, u, dist+topk (psum pool) ----
                    with tc.tile_pool(name=f"l{li}p1", bufs=2, space="PSUM") as pp1:
                        for pt in range(P // 128):
                            vps = pp1.tile([128, DH], F32, name="vps", tag="vps")
                            for ci in range(NDC):
                                nc.tensor.matmul(
                                    vps[:], xTr_tiles[ci][:, pt * 128:(pt + 1) * 128],
                                    Wb[ci][:], start=(ci == 0), stop=(ci == NDC - 1))
                            vrow = wp.tile([128, DH], F32, name="vrow", tag="vrow")
                            nc.scalar.activation(vrow[:], vps[:], AF.Copy)
                            nc.sync.dma_start(v_dram[pt * 128:(pt + 1) * 128, :], vrow[:])

                        urow_tiles = []
                        for pt in range(NQ // 128):
                            ups = pp1.tile([128, DH], F32, name="ups", tag="vps")
                            for ci in range(NDC):
                                nc.tensor.matmul(
                                    ups[:], QTr[ci][:, pt * 128:(pt + 1) * 128],
                                    Wd[ci][:], start=(ci == 0), stop=False)
                            nc.tensor.matmul(ups[:],
                                             onesr_r[:, pt * 128:(pt + 1) * 128],
                                             Wba[0][:], start=False, stop=True)
                            ur = lp.tile([128, DH], F32, name=f"urow{pt}")
                            nc.scalar.activation(ur[:], ups[:], AF.Copy)
                            urow_tiles.append(ur)

                        idx16 = lp.tile([128, 8 * K], mybir.dt.int16, name="idx16")
                        score = lp.tile([128, P], F32, name="score")
                        maxv = lp.tile([128, 24], F32, name="maxv")
                        idxs = lp.tile([128, 24], mybir.dt.uint32, name="idxs")
                        for t in range(NQ // 128):
                            for hb in range(2):
                                dps = pp1.tile([128, 1024], F32, name="dps", tag="dps")
                                for nb in range(2):
                                    sl = slice(hb * 1024 + nb * 512,
                                               hb * 1024 + (nb + 1) * 512)
                                    psl = slice(nb * 512, (nb + 1) * 512)
                                    for ci in range(NDC):
                                        nc.tensor.matmul(
                                            dps[:, psl],
                                            QT[ci][:, t * 128:(t + 1) * 128],
                                            xT_tiles[ci][:, sl],
                                            start=(ci == 0), stop=(ci == NDC - 1))
                                nc.vector.tensor_tensor(
                                    score[:, hb * 1024:(hb + 1) * 1024], dps[:],
                                    nsqb[:, hb * 1024:(hb + 1) * 1024], op=ALU.add)
                            for r in range(3):
                                nc.vector.max(maxv[:, 8 * r:8 * r + 8], score[:])
                                nc.vector.max_index(idxs[:, 8 * r:8 * r + 8],
                                                    maxv[:, 8 * r:8 * r + 8], score[:])
                                if r < 2:
                                    nc.vector.match_replace(
                                        score[:], maxv[:, 8 * r:8 * r + 8], score[:], NEG)
                            nc.vector.tensor_copy(idx16[:, t * K:(t + 1) * K],
                                                  idxs[:, :K])

                    # ---- wrap indices ----
                    nc.sync.dma_start(idx_dram[:, :], idx16[:])
                    wrapped = lp.tile([128, 8 * K * 8], mybir.dt.int16, name="wrapped")
                    vsrc = idx_dram[:, :].rearrange("(r q) tc -> q tc r", r=8, q=16)
                    for bb_ in range(8):
                        nc.sync.dma_start(
                            wrapped[bb_ * 16:(bb_ + 1) * 16, :].rearrange(
                                "q (tc r) -> q tc r", r=8),
                            vsrc)

                    # ---- phase 2: edge MLP + max over k ----
                    gacc = [lp.tile([128, 512], F32, name=f"acc{g}_{m}")
                            for g in range(2) for m in range(NMC)]
                    with tc.tile_pool(name=f"l{li}p2", bufs=8, space="PSUM") as pp2:
                        for g in range(2):
                            tiles4 = [g * 4 + i for i in range(4)]
                            for c0 in range(0, K, CCHUNK):
                                vk = {}
                                for tt in tiles4:
                                    vkt = wp.tile([128, CCHUNK, DH], F32,
                                                  name=f"vk{tt % 4}", tag=f"vk{tt % 4}")
                                    nc.gpsimd.dma_gather(
                                        out_ap=vkt[:], in_ap=v_dram[:, :],
                                        idxs_ap=wrapped[:, tt * K * 8 + c0 * 8:
                                                        tt * K * 8 + (c0 + CCHUNK) * 8],
                                        num_idxs=128 * CCHUNK,
                                        num_idxs_reg=128 * CCHUNK,
                                        elem_size=DH, queue_num=tt % 4)
                                    vk[tt] = vkt
                                for kk in range(CCHUNK):
                                    h1 = []
                                    for h in range(NHC):
                                        rows = min(128, DH - h * 128)
                                        hps = pp2.tile([rows, 512], F32,
                                                       name="hps", tag="bank")
                                        for j, tt in enumerate(tiles4):
                                            nc.tensor.matmul(
                                                hps[:, j * 128:(j + 1) * 128],
                                                vk[tt][:, kk, h * 128:h * 128 + rows],
                                                ident[:], is_transpose=True,
                                                start=(j == 0), stop=False)
                                            nc.tensor.matmul(
                                                hps[:, j * 128:(j + 1) * 128],
                                                urow_tiles[tt][:, h * 128:h * 128 + rows],
                                                ident[:], is_transpose=True,
                                                start=False, stop=(j == 3))
                                        h1t = wp.tile([rows, 512], F32R,
                                                      name=f"h1t{h}", tag=f"h1t{h}")
                                        nc.scalar.activation(h1t[:], hps[:], AF.Relu)
                                        h1.append(h1t)
                                    for m in range(NMC):
                                        mrows = min(128, DO - m * 128)
                                        h2ps = pp2.tile([mrows, 512], F32,
                                                        name="h2ps", tag="bank")
                                        for h in range(NHC):
                                            nc.tensor.matmul(
                                                h2ps[:],
                                                Wwb[h][:, m * 128:m * 128 + mrows],
                                                h1[h][:],
                                                start=(h == 0), stop=(h == NHC - 1))
                                        am = gacc[g * NMC + m]
                                        if c0 == 0 and kk == 0:
                                            nc.vector.tensor_copy(am[:mrows, :], h2ps[:])
                                        else:
                                            nc.vector.tensor_tensor(
                                                am[:mrows, :], h2ps[:], am[:mrows, :],
                                                op=ALU.max)

                    # ---- relu(acc + bb) -> own xT block -> DRAM ----
                    for m in range(NMC):
                        mrows = min(128, DO - m * 128)
                        for g in range(2):
                            xo = wp.tile([128, 512], F32, name="xo", tag="xo")
                            nc.scalar.activation(
                                xo[:mrows, :], gacc[g * NMC + m][:mrows, :],
                                AF.Relu, bias=Wbb[m][:mrows, :])
                            nc.sync.dma_start(
                                xown_dram[li - 1][m * 128:m * 128 + mrows,
                                                  g * 512:(g + 1) * 512],
                                xo[:mrows, :])

                    if li < 4:
                        nc.gpsimd.collective_compute(
                            "AllGather", ALU.bypass, replica_groups=groups,
                            ins=[xown_dram[li - 1].opt()],
                            outs=[ag_out[li - 1].opt()])

                if li < 4:
                    xp = xping if li % 2 == 1 else xpong
                    with tc.tile_pool(name=f"x{li}ps", bufs=1, space="PSUM") as xpp, \
                         tc.tile_pool(name=f"x{li}tmp", bufs=1) as xtmp:
                        xT_tiles = []
                        for m in range(NMC):
                            mrows = min(128, DO - m * 128)
                            xt = xp.tile([mrows, P], F32, name=f"xTn{li}_{m}",
                                         tag=f"xTn{li}_{m}")
                            nc.sync.dma_start(
                                xt[:, 0:NQ],
                                ag_out[li - 1][m * 128:m * 128 + mrows, :])
                            nc.sync.dma_start(
                                xt[:, NQ:P],
                                ag_out[li - 1][DO + m * 128:DO + m * 128 + mrows, :])
                            xT_tiles.append(xt)
                        sq = xtmp.tile([128, P], F32, name="sqtmp")
                        sps = xpp.tile([1, P], F32, name="sps", space="PSUM")
                        for m in range(NMC):
                            mrows = min(128, DO - m * 128)
                            nc.vector.tensor_tensor(sq[:mrows, :], xT_tiles[m][:],
                                                    xT_tiles[m][:], op=ALU.mult)
                            for nb in range(P // 512):
                                nc.tensor.matmul(
                                    sps[:, nb * 512:(nb + 1) * 512],
                                    onescol[:mrows, :],
                                    sq[:mrows, nb * 512:(nb + 1) * 512],
                                    start=(m == 0), stop=(m == NMC - 1))
                        nsqrow = xtmp.tile([1, P], F32, name="nsqrow")
                        nc.scalar.activation(nsqrow[:], sps[:], AF.Copy, scale=-0.5)
                        nc.sync.dma_start(nsq_dram[li][0:1, :], nsqrow[:])

            # ---------------- FC head ----------------
            with tc.tile_pool(name="fc", bufs=1) as fp, \
                 tc.tile_pool(name="fcw", bufs=1) as fw, \
                 tc.tile_pool(name="fcps", bufs=2, space="PSUM") as fpp:
                feat_chunks = [64, 128, 128, 128, 128, 128, 128, 128]
                Wf1 = load_round(fw, "fw1", (960, 512), row_chunks=feat_chunks)
                Wfb1 = load_round(fw, "fb1", (1, 512))
                Wf2 = load_round(fw, "fw2", (512, 256))
                Wfb2 = load_round(fw, "fb2", (1, 256))
                Wf3 = load_round(fw, "fw3", (256, 1))
                Wfb3 = load_round(fw, "fb3", (1, 1))

                feat_r = []
                for li, (_, _, do) in enumerate(LDIMS, start=1):
                    for m in range(cdiv(do, 128)):
                        mrows = min(128, do - m * 128)
                        f32t = fp.tile([mrows, NQ], F32, name=f"ff{li}_{m}")
                        nc.sync.dma_start(
                            f32t[:], xown_dram[li - 1][m * 128:m * 128 + mrows, :])
                        frt = fp.tile([mrows, NQ], F32R, name=f"fr{li}_{m}")
                        nc.vector.tensor_copy(frt[:], f32t[:])
                        feat_r.append(frt)

                def fc_layer(rhs_chunks, wtiles, btile, nout, act):
                    outs = []
                    for m in range(cdiv(nout, 128)):
                        mrows = min(128, nout - m * 128)
                        ot = fp.tile([mrows, NQ], F32R if act == AF.Relu else F32,
                                     name=f"fco{nout}_{m}")
                        for g in range(2):
                            ps = fpp.tile([mrows, 512], F32, name="fps", tag="fcps",
                                          space="PSUM")
                            for ci, rc in enumerate(rhs_chunks):
                                nc.tensor.matmul(
                                    ps[:], wtiles[ci][:, m * 128:m * 128 + mrows],
                                    rc[:, g * 512:(g + 1) * 512],
                                    start=(ci == 0), stop=False)
                            nc.tensor.matmul(
                                ps[:], btile[0][:, m * 128:m * 128 + mrows],
                                onesr_r[:, g * 512:(g + 1) * 512],
                                start=False, stop=True)
                            nc.scalar.activation(ot[:, g * 512:(g + 1) * 512],
                                                 ps[:], act)
                        outs.append(ot)
                    return outs

                h1fc = fc_layer(feat_r, Wf1, Wfb1, 512, AF.Relu)
                h2fc = fc_layer(h1fc, Wf2, Wfb2, 256, AF.Relu)
                yt = fc_layer(h2fc, Wf3, Wfb3, 1, AF.Sigmoid)
                nc.sync.dma_start(y_out[:, :], yt[0][:])

    nc.compile()
    return nc


def kernel(**inputs):
    x = np.asarray(inputs["x"], np.float32)          # [8192, 3]
    if "nc" not in _CACHED:
        _CACHED["nc"] = _build()
    nc = _CACHED["nc"]

    base = {}
    for li in range(1, 5):
        wa = np.asarray(inputs[f"w{li}a"], np.float32)
        D = wa.shape[0] // 2
        base[f"wdiff{li}"] = np.ascontiguousarray(wa[:D] - wa[D:])
        base[f"wbot{li}"] = np.ascontiguousarray(wa[D:])
        base[f"ba{li}"] = np.asarray(inputs[f"b{li}a"], np.float32)[None, :]
        base[f"wb{li}"] = np.asarray(inputs[f"w{li}b"], np.float32)
        base[f"bb{li}"] = np.asarray(inputs[f"b{li}b"], np.float32)[:, None]
    for nm in ("fw1", "fw2", "fw3"):
        base[nm] = np.asarray(inputs[nm], np.float32)
    for nm in ("fb1", "fb2", "fb3"):
        base[nm] = np.asarray(inputs[nm], np.float32)[None, :]

    in_maps = []
    for c in range(N_CORES):
        cloud = c // 2
        xc = x[cloud * P:(cloud + 1) * P]
        m = dict(base)
        m["xT"] = np.ascontiguousarray(xc.T)
        m["nsq"] = (-0.5 * (xc * xc).sum(1))[None, :].astype(np.float32)
        in_maps.append(m)

    res = run_bass_kernel_spmd(nc, in_maps, core_ids=list(range(N_CORES)))
    out = np.empty((B * P, 1), np.float32)
    for c in range(N_CORES):
        cloud, half = c // 2, c % 2
        out[cloud * P + half * NQ: cloud * P + (half + 1) * NQ, 0] = \
            res.results[c]["y"][0]
    return out



# revision 16
# speedup vs baseline: 1.2665x; 1.2665x over previous
"""DGCNN (4x EdgeConv + FC head) Bass kernel for 8 trn2 NeuronCores — v2.

Sharding: cloud b -> cores {2b, 2b+1}; each core owns 1024 query points
(q0 = (pid % 2) * 1024). Full cloud features exchanged within each pair via
bf16 AllGather after layers 1-3.

Design vs v1 baseline:
- bf16 compute throughout (PE 1 cyc/row incl. transposes; halved DMA bytes).
- Transposed dma_gather delivers neighbor features feature-major: no PE
  transposes in the edge MLP. Gathers raw x_j (dim D), h1 computed directly
  as wbot^T x_j + u_i via PE (u injected with a selector matmul).
- Top-k via composite packing: u32 = (bf16(score) << 16) | idx. Segment Max8
  (8x256) + 3 merge rounds on 64 candidates; no full-width MaxIndex scans.
- -|xj|^2/2 folded into the dist matmul as two bf16 hi/lo rows.
- max over K neighbors via single tensor_reduce per (qh, m) from PSUM.
- PSUM: dist 2 banks + h1 2 + h2acc <=4 = 8, allows cross-tile pipelining.
"""
import numpy as np
import ml_dtypes

import concourse.bass as bass
import concourse.bacc as bacc
import concourse.mybir as mybir
import concourse.tile as tile
from concourse.bass_utils import run_bass_kernel_spmd

B, P, K = 4, 2048, 20
NQ = 1024
N_CORES = 8
NEG = -3.0e38
F32 = mybir.dt.float32
BF16 = mybir.dt.bfloat16
U32 = mybir.dt.uint32
U16 = mybir.dt.uint16
I16 = mybir.dt.int16
AF = mybir.ActivationFunctionType
ALU = mybir.AluOpType
AX = mybir.AxisListType

#          D  Dpad  DH   DO
LCFG = [(3, 128, 64, 64),
        (64, 128, 128, 128),
        (128, 128, 256, 256),
        (256, 256, 512, 512)]
FC1_CHUNKS = [64, 128, 128, 128, 128, 128, 128, 128]  # 960 rows

_CACHED = {}


def cdiv(a, b):
    return (a + b - 1) // b


def _build():
    nc = bacc.Bacc("TRN2", target_bir_lowering=False, debug=False,
                   num_devices=N_CORES, num_swdge_queues=4)

    # ---------------- DRAM params ----------------
    xTb1_in = nc.declare_dram_parameter("xTb1", [3, P], BF16, isOutput=False)
    xsb1_in = nc.declare_dram_parameter("xsb1", [128, 16 * 128], BF16, isOutput=False)
    nsq1_in = nc.declare_dram_parameter("nsq1", [2, P], BF16, isOutput=False)
    selI_in = nc.declare_dram_parameter("selI", [16, 320], BF16, isOutput=False)
    wp = {}
    for li, (D, DP, DH, DO) in enumerate(LCFG, start=1):
        wp[f"wdiff{li}"] = nc.declare_dram_parameter(f"wdiff{li}", [D, DH], BF16, isOutput=False)
        wp[f"wbot{li}"] = nc.declare_dram_parameter(f"wbot{li}", [DP, DH], BF16, isOutput=False)
        wp[f"ba{li}"] = nc.declare_dram_parameter(f"ba{li}", [2, DH], BF16, isOutput=False)
        wp[f"wb{li}"] = nc.declare_dram_parameter(f"wb{li}", [DH, DO], BF16, isOutput=False)
        wp[f"bb{li}"] = nc.declare_dram_parameter(f"bb{li}", [DO, 1], F32, isOutput=False)
    wp["fw1"] = nc.declare_dram_parameter("fw1", [960, 512], BF16, isOutput=False)
    wp["fb1"] = nc.declare_dram_parameter("fb1", [512, 1], F32, isOutput=False)
    wp["fw2"] = nc.declare_dram_parameter("fw2", [512, 256], BF16, isOutput=False)
    wp["fb2"] = nc.declare_dram_parameter("fb2", [256, 1], F32, isOutput=False)
    wp["fw3"] = nc.declare_dram_parameter("fw3", [256, 1], BF16, isOutput=False)
    wp["fb3"] = nc.declare_dram_parameter("fb3", [1, 1], F32, isOutput=False)
    y_out = nc.declare_dram_parameter("y", [1, NQ], F32, isOutput=True)

    groups = [[2 * b, 2 * b + 1] for b in range(N_CORES // 2)]

    with tile.TileContext(nc) as tc:
        with tc.tile_pool(name="const", bufs=1) as cp, \
             tc.tile_pool(name="glob", bufs=1) as gp, \
             tc.tile_pool(name="dram", bufs=1, space="DRAM") as dram:

            q0 = nc.vector.partition_id()
            q0 = nc.vector.scalar_reg_alu(ALU.mod, q0, 2)
            q0 = nc.vector.scalar_reg_alu(ALU.mult, q0, NQ)

            selI = cp.tile([16, 320], BF16, name="selI")
            nc.sync.dma_start(selI[:], selI_in[:, :])
            ones2 = cp.tile([2, 128], BF16, name="ones2")
            nc.vector.memset(ones2[:], 1.0)
            onescol = cp.tile([128, 1], BF16, name="onescol")
            nc.vector.memset(onescol[:], 1.0)

            # weights -> SBUF (bf16, direct DMA)
            W = {}
            for li, (D, DP, DH, DO) in enumerate(LCFG, start=1):
                for nm, rows in ((f"wdiff{li}", D), (f"wbot{li}", DP), (f"wb{li}", DH)):
                    tiles = []
                    for c0 in range(0, rows, 128):
                        r = min(128, rows - c0)
                        t = cp.tile([r, wp[nm].shape[1]], BF16, name=f"{nm}_{c0}")
                        nc.sync.dma_start(t[:], wp[nm][c0:c0 + r, :])
                        tiles.append(t)
                    W[nm] = tiles
                t = cp.tile([2, DH], BF16, name=f"ba{li}")
                nc.sync.dma_start(t[:], wp[f"ba{li}"][:, :])
                W[f"ba{li}"] = t
                tiles = []
                for c0 in range(0, DO, 128):
                    r = min(128, DO - c0)
                    t = cp.tile([r, 1], F32, name=f"bb{li}_{c0}")
                    nc.sync.dma_start(t[:], wp[f"bb{li}"][c0:c0 + r, :])
                    tiles.append(t)
                W[f"bb{li}"] = tiles
            fw1_tiles = []
            r0 = 0
            for ci, r in enumerate(FC1_CHUNKS):
                t = cp.tile([r, 512], BF16, name=f"fw1_{ci}")
                nc.sync.dma_start(t[:], wp["fw1"][r0:r0 + r, :])
                fw1_tiles.append(t)
                r0 += r
            fw2_tiles = []
            for c0 in range(0, 512, 128):
                t = cp.tile([128, 256], BF16, name=f"fw2_{c0}")
                nc.sync.dma_start(t[:], wp["fw2"][c0:c0 + 128, :])
                fw2_tiles.append(t)
            fw3_tiles = []
            for c0 in range(0, 256, 128):
                t = cp.tile([128, 1], BF16, name=f"fw3_{c0}")
                nc.sync.dma_start(t[:], wp["fw3"][c0:c0 + 128, :])
                fw3_tiles.append(t)
            fbs = {}
            for nm, w in (("fb1", 512), ("fb2", 256), ("fb3", 1)):
                tiles = []
                for c0 in range(0, w, 128):
                    r = min(128, w - c0)
                    t = cp.tile([r, 1], F32, name=f"{nm}_{c0}")
                    nc.sync.dma_start(t[:], wp[nm][c0:c0 + r, :])
                    tiles.append(t)
                fbs[nm] = tiles

            # persistent double-buffered per-tile structures
            comp = []
            for i in range(2):
                t = gp.tile([128, P], U32, name=f"comp{i}")
                nc.gpsimd.iota(t[:], [[1, P]], base=0, channel_multiplier=0)
                comp.append(t)
            wrapped = []
            for i in range(2):
                t = gp.tile([128, 8 * K], I16, name=f"wrap{i}")
                nc.vector.memset(t[:], 0)
                wrapped.append(t)
            scoreb = [gp.tile([128, P], BF16, name=f"scoreb{i}") for i in range(2)]
            segtop = [gp.tile([128, 64], F32, name=f"segtop{i}") for i in range(2)]
            top24 = [gp.tile([128, 24], F32, name=f"top24{i}") for i in range(2)]
            idx16 = [gp.tile([128, 24], I16, name=f"idx16{i}") for i in range(2)]
            idx_dram = [dram.tile([128, K], I16, name=f"idxd{i}") for i in range(2)]

            # resident per-layer outputs (feature-major) for the FC head
            xoT = {}
            for li, (_, _, _, DO) in enumerate(LCFG, start=1):
                xoT[li] = [gp.tile([min(128, DO - c0), NQ], BF16,
                                   name=f"xoT{li}_{c0}")
                           for c0 in range(0, DO, 128)]

            ag_in = [dram.tile([do, NQ], BF16, name=f"agin{li}")
                     for li, (_, _, _, do) in enumerate(LCFG[:3], start=1)]
            ag_out = [dram.tile([2 * do, NQ], BF16, name=f"agout{li}")
                      for li, (_, _, _, do) in enumerate(LCFG[:3], start=1)]

            xTb = None     # list of [<=128, P] bf16 feature-major chunks
            nsq2 = None    # [2, P] bf16 hi/lo of -0.5|x|^2

            for li, (D, DP, DH, DO) in enumerate(LCFG, start=1):
                NDC = cdiv(D, 128)     # unpadded contract chunks (dist, u)
                NDCP = DP // 128       # padded contract chunks (gather/h1)
                NHC = cdiv(DH, 128)
                NMC = cdiv(DO, 128)
                h2_bufs = 2 if NMC <= 2 else 1

                with tc.tile_pool(name=f"l{li}", bufs=1) as lp, \
                     tc.tile_pool(name=f"l{li}w", bufs=2) as wkp, \
                     tc.tile_pool(name=f"l{li}ps", bufs=2, space="PSUM") as pdist, \
                     tc.tile_pool(name=f"l{li}h1", bufs=2, space="PSUM") as ph1, \
                     tc.tile_pool(name=f"l{li}h2", bufs=h2_bufs, space="PSUM") as ph2:

                    # ---- layer inputs: xTb (feature-major), xsb (point-major) ----
                    if li == 1:
                        t = lp.tile([3, P], BF16, name="xTb1")
                        nc.sync.dma_start(t[:], xTb1_in[:, :])
                        xTb = [t]
                        xsb = lp.tile([128, 16, 128], BF16, name="xsb1")
                        nc.sync.dma_start(
                            xsb[:].rearrange("p r d -> p (r d)"), xsb1_in[:, :])
                        nsq2 = lp.tile([2, P], BF16, name="nsq1")
                        nc.sync.dma_start(nsq2[:], nsq1_in[:, :])
                    else:
                        DPREV = LCFG[li - 2][3]
                        xTb = []
                        for c0 in range(0, DPREV, 128):
                            r = min(128, DPREV - c0)
                            t = lp.tile([r, P], BF16, name=f"xTb{li}_{c0}")
                            nc.sync.dma_start(t[:, 0:NQ],
                                              ag_out[li - 2][c0:c0 + r, :])
                            nc.sync.dma_start(t[:, NQ:P],
                                              ag_out[li - 2][DPREV + c0:DPREV + c0 + r, :])
                            xTb.append(t)
                        xsb = lp.tile([128, 16, DP], BF16, name=f"xsb{li}")
                        if DPREV < DP:
                            nc.vector.memset(xsb[:, :, DPREV:DP], 0.0)
                        for ci, xt in enumerate(xTb):
                            nc.scalar.dma_start_transpose(
                                xsb[:, :, ci * 128:ci * 128 + xt.shape[0]], xt[:])
                        # nsq2 = hi/lo bf16 of -0.5 * sum_f x^2
                        nsq2 = lp.tile([2, P], BF16, name=f"nsq{li}")
                        nsqlo = lp.tile([1, P], BF16, name=f"nsqlo{li}")
                        sqb = lp.tile([128, P], BF16, name=f"sqb{li}")
                        for nb in range(4):
                            nsqps = pdist.tile([128, 512], F32, name="nsqps", tag="dps")
                            for ci, xt in enumerate(xTb):
                                r = xt.shape[0]
                                sl = slice(nb * 512, (nb + 1) * 512)
                                nc.vector.tensor_tensor(sqb[:r, sl], xt[:, sl],
                                                        xt[:, sl], op=ALU.mult)
                                nc.tensor.matmul(nsqps[0:1, :], onescol[:r, :],
                                                 sqb[:r, sl], start=(ci == 0),
                                                 stop=(ci == len(xTb) - 1))
                            nc.scalar.activation(nsq2[0:1, nb * 512:(nb + 1) * 512],
                                                 nsqps[0:1, :], AF.Copy, scale=-0.5)
                            nc.vector.scalar_tensor_tensor(
                                nsqlo[0:1, nb * 512:(nb + 1) * 512], nsqps[0:1, :],
                                -0.5, nsq2[0:1, nb * 512:(nb + 1) * 512],
                                op0=ALU.mult, op1=ALU.subtract)
                        nc.sync.dma_start(nsq2[1:2, :], nsqlo[0:1, :])

                    # ---- query-slice copies (static offsets for lhsT) ----
                    xq = []
                    for ci, xt in enumerate(xTb):
                        r = xt.shape[0]
                        qt = lp.tile([r, NQ], BF16, name=f"xq{ci}")
                        nc.vector.tensor_copy(qt[:], xt[:, bass.ds(q0, NQ)])
                        xq.append(qt)

                    # ---- u tiles: u = xq @ wdiff + ba, folded to [16, 8, DH] ----
                    usb = []
                    for t in range(8):
                        ups = pdist.tile([128, 512], F32, name="ups", tag="dps")
                        tsl = slice(t * 128, (t + 1) * 128)
                        for ci in range(NDC):
                            nc.tensor.matmul(ups[:, :DH], xq[ci][:, tsl],
                                             W[f"wdiff{li}"][ci][:],
                                             start=(ci == 0), stop=False)
                        nc.tensor.matmul(ups[:, :DH], ones2[:],
                                         W[f"ba{li}"][:], start=False, stop=True)
                        ut = lp.tile([128, DH], BF16, name=f"ust{t}", tag="ust",
                                     bufs=2)
                        nc.scalar.activation(ut[:], ups[:, :DH], AF.Copy)
                        ud = dram.tile([128, DH], BF16, name=f"ud{li}_{t}")
                        nc.sync.dma_start(ud[:, :], ut[:])
                        uq = lp.tile([16, 8, DH], BF16, name=f"usb{t}")
                        nc.sync.dma_start(
                            uq[:], ud[:, :].rearrange("(qh ql) d -> ql qh d", ql=16))
                        usb.append(uq)

                    macc = [lp.tile([min(128, DO - c0), NQ], F32,
                                    name=f"macc{li}_{c0}")
                            for c0 in range(0, DO, 128)]

                    # ---- main per-tile loop ----
                    for t in range(8):
                        tb = t % 2
                        tsl = slice(t * 128, (t + 1) * 128)
                        # dist quarters -> score bf16
                        for nb in range(4):
                            dps = pdist.tile([128, 512], F32, name="dps", tag="dps")
                            sl = slice(nb * 512, (nb + 1) * 512)
                            for ci in range(NDC):
                                nc.tensor.matmul(dps[:], xq[ci][:, tsl],
                                                 xTb[ci][:, sl],
                                                 start=(ci == 0), stop=False)
                            nc.tensor.matmul(dps[:], ones2[:], nsq2[:, sl],
                                             start=False, stop=True)
                            nc.scalar.activation(scoreb[tb][:, sl], dps[:], AF.Copy)
                        # composite topk
                        cb = comp[tb]
                        nc.vector.tensor_copy(cb[:].bitcast(U16)[:, 1::2],
                                              scoreb[tb][:].bitcast(U16))
                        compf = cb[:].bitcast(F32)
                        st = segtop[tb]
                        for s in range(8):
                            nc.vector.max(st[:, s * 8:(s + 1) * 8],
                                          compf[:, s * 256:(s + 1) * 256])
                        t24 = top24[tb]
                        for r in range(3):
                            nc.vector.max(t24[:, 8 * r:8 * r + 8], st[:])
                            if r < 2:
                                nc.vector.match_replace(
                                    st[:], t24[:, 8 * r:8 * r + 8], st[:], NEG)
                        nc.vector.tensor_copy(idx16[tb][:],
                                              t24[:].bitcast(I16)[:, 0::2])
                        # wrap indices: dram bounce + 8-block replication
                        nc.sync.dma_start(idx_dram[tb][:, :], idx16[tb][:, 0:K])
                        wsrc = idx_dram[tb][:, :].rearrange(
                            "(qh ql) k -> ql qh k", ql=16)
                        for bb in range(8):
                            nc.sync.dma_start(
                                wrapped[tb][bb * 16:(bb + 1) * 16, :].rearrange(
                                    "ql (qh k) -> ql qh k", k=K), wsrc)
                        # transposed gathers: 4 chunks x 640 edges (2 qh each)
                        vkc = []
                        for g in range(4):
                            vt = wkp.tile([128, NDCP, 640], BF16,
                                          name=f"vk{g}", tag=f"vk{g}")
                            nc.gpsimd.dma_gather(
                                out_ap=vt[:], in_ap=xsb[:].rearrange("p r d -> p (r d)"),
                                idxs_ap=wrapped[tb][:, g * 40:(g + 1) * 40],
                                num_idxs=640, num_idxs_reg=640, elem_size=DP,
                                transpose=True, queue_num=g,
                                sbuf_tokens_per_rank=128,
                                sbuf_free_dim_per_rank=DP * 2,
                                sbuf_free_dim_pad_per_rank=0,
                                sbuf_byte_offset=0)
                            vkc.append(vt)
                        # edge MLP per qh block (320 edges)
                        for qh in range(8):
                            vt = vkc[qh // 2]
                            off = (qh % 2) * 320
                            h1sb = wkp.tile([128, NHC, 320], BF16,
                                            name="h1sb", tag="h1sb")
                            for hc in range(NHC):
                                hr = min(128, DH - hc * 128)
                                h1ps = ph1.tile([128, 320], F32, name="h1ps",
                                                tag="h1ps")
                                for dc in range(NDCP):
                                    nc.tensor.matmul(
                                        h1ps[:hr, :],
                                        W[f"wbot{li}"][dc][:, hc * 128:hc * 128 + hr],
                                        vt[:, dc, off:off + 320],
                                        start=(dc == 0), stop=False)
                                nc.tensor.matmul(
                                    h1ps[:hr, :],
                                    usb[t][:, qh, hc * 128:hc * 128 + hr],
                                    selI[:], start=False, stop=True)
                                nc.scalar.activation(h1sb[:hr, hc, :],
                                                     h1ps[:hr, :], AF.Relu)
                            for m in range(NMC):
                                mr = min(128, DO - m * 128)
                                h2ps = ph2.tile([128, 320], F32, name="h2ps",
                                                tag=f"h2_{m}")
                                for hc in range(NHC):
                                    hr = min(128, DH - hc * 128)
                                    nc.tensor.matmul(
                                        h2ps[:mr, :],
                                        W[f"wb{li}"][hc][:hr, m * 128:m * 128 + mr],
                                        h1sb[:hr, hc, :],
                                        start=(hc == 0), stop=(hc == NHC - 1))
                                nc.vector.tensor_reduce(
                                    macc[m][:mr, t * 128 + qh * 16:
                                            t * 128 + qh * 16 + 16],
                                    h2ps[:mr, :].rearrange("p (k ql) -> p ql k",
                                                           k=K),
                                    axis=AX.X, op=ALU.max)

                    # ---- xo = relu(macc + bb) -> xoT (+ AllGather input) ----
                    for m in range(NMC):
                        mr = min(128, DO - m * 128)
                        nc.scalar.activation(xoT[li][m][:], macc[m][:mr, :],
                                             AF.Relu, bias=W[f"bb{li}"][m][:])
                        if li < 4:
                            nc.sync.dma_start(
                                ag_in[li - 1][m * 128:m * 128 + mr, :],
                                xoT[li][m][:])
                    if li < 4:
                        nc.gpsimd.collective_compute(
                            "AllGather", ALU.bypass, replica_groups=groups,
                            ins=[ag_in[li - 1].opt()],
                            outs=[ag_out[li - 1].opt()])

            # ---------------- FC head ----------------
            with tc.tile_pool(name="fc", bufs=1) as fp, \
                 tc.tile_pool(name="fcps", bufs=2, space="PSUM") as fpp:
                feats = [xoT[1][0], xoT[2][0], xoT[3][0], xoT[3][1],
                         xoT[4][0], xoT[4][1], xoT[4][2], xoT[4][3]]
                h1fc = [fp.tile([128, NQ], BF16, name=f"h1fc{m}") for m in range(4)]
                for m in range(4):
                    for g in range(2):
                        ps = fpp.tile([128, 512], F32, name="fps", tag="fps")
                        gsl = slice(g * 512, (g + 1) * 512)
                        for ci, ft in enumerate(feats):
                            nc.tensor.matmul(ps[:],
                                             fw1_tiles[ci][:, m * 128:(m + 1) * 128],
                                             ft[:, gsl],
                                             start=(ci == 0), stop=(ci == 7))
                        nc.scalar.activation(h1fc[m][:, gsl], ps[:], AF.Relu,
                                             bias=fbs["fb1"][m][:])
                h2fc = [fp.tile([128, NQ], BF16, name=f"h2fc{m}") for m in range(2)]
                for m in range(2):
                    for g in range(2):
                        ps = fpp.tile([128, 512], F32, name="fps2", tag="fps")
                        gsl = slice(g * 512, (g + 1) * 512)
                        for ci in range(4):
                            nc.tensor.matmul(ps[:],
                                             fw2_tiles[ci][:, m * 128:(m + 1) * 128],
                                             h1fc[ci][:, gsl],
                                             start=(ci == 0), stop=(ci == 3))
                        nc.scalar.activation(h2fc[m][:, gsl], ps[:], AF.Relu,
                                             bias=fbs["fb2"][m][:])
                yt = fp.tile([1, NQ], F32, name="yt")
                for g in range(2):
                    ps = fpp.tile([1, 512], F32, name="fps3", tag="fps3")
                    gsl = slice(g * 512, (g + 1) * 512)
                    for ci in range(2):
                        nc.tensor.matmul(ps[:], fw3_tiles[ci][:],
                                         h2fc[ci][:, gsl],
                                         start=(ci == 0), stop=(ci == 1))
                    nc.scalar.activation(yt[:, gsl], ps[:], AF.Sigmoid,
                                         bias=fbs["fb3"][0][:])
                nc.sync.dma_start(y_out[:, :], yt[:])

    nc.compile()
    return nc


def _bf16(a):
    return np.asarray(a, np.float32).astype(ml_dtypes.bfloat16)


def _hilo(row):
    """f32 row -> [2, N] bf16 (hi, residual)."""
    hi = row.astype(ml_dtypes.bfloat16)
    lo = (row - hi.astype(np.float32)).astype(ml_dtypes.bfloat16)
    return np.stack([hi.astype(np.float32), lo.astype(np.float32)]).astype(
        ml_dtypes.bfloat16)


def kernel(**inputs):
    x = np.asarray(inputs["x"], np.float32)          # [8192, 3]
    if "nc" not in _CACHED:
        _CACHED["nc"] = _build()
    nc = _CACHED["nc"]

    selI = np.zeros((16, 320), np.float32)
    for k in range(K):
        for ql in range(16):
            selI[ql, k * 16 + ql] = 1.0

    base = {"selI": selI.astype(ml_dtypes.bfloat16)}
    for li, (D, DP, DH, DO) in enumerate(LCFG, start=1):
        wa = np.asarray(inputs[f"w{li}a"], np.float32)
        wtop, wbot = wa[:D], wa[D:]
        base[f"wdiff{li}"] = _bf16(wtop - wbot)
        wbp = np.zeros((DP, DH), np.float32)
        wbp[:D] = wbot
        base[f"wbot{li}"] = _bf16(wbp)
        base[f"ba{li}"] = _hilo(np.asarray(inputs[f"b{li}a"], np.float32))
        base[f"wb{li}"] = _bf16(inputs[f"w{li}b"])
        base[f"bb{li}"] = np.asarray(inputs[f"b{li}b"], np.float32)[:, None]
    base["fw1"] = _bf16(inputs["fw1"])
    base["fb1"] = np.asarray(inputs["fb1"], np.float32)[:, None]
    base["fw2"] = _bf16(inputs["fw2"])
    base["fb2"] = np.asarray(inputs["fb2"], np.float32)[:, None]
    base["fw3"] = _bf16(inputs["fw3"])
    base["fb3"] = np.asarray(inputs["fb3"], np.float32)[:, None]

    in_maps = []
    for c in range(N_CORES):
        cloud = c // 2
        xc = x[cloud * P:(cloud + 1) * P]
        m = dict(base)
        m["xTb1"] = _bf16(xc.T)
        xp = np.zeros((P, 128), np.float32)
        xp[:, :3] = xc
        m["xsb1"] = _bf16(
            xp.reshape(16, 128, 128).transpose(1, 0, 2).reshape(128, 16 * 128))
        m["nsq1"] = _hilo(-0.5 * (xc * xc).sum(1))
        in_maps.append(m)

    res = run_bass_kernel_spmd(nc, in_maps, core_ids=list(range(N_CORES)))
    out = np.empty((B * P, 1), np.float32)
    for c in range(N_CORES):
        cloud, half = c // 2, c % 2
        out[cloud * P + half * NQ: cloud * P + (half + 1) * NQ, 0] = \
            res.results[c]["y"][0]
    return out


# revision 27
# speedup vs baseline: 1857.3673x; 1466.5455x over previous
"""DGCNN (4x EdgeConv + FC head) Bass kernel for 8 trn2 NeuronCores — v2.

Sharding: cloud b -> cores {2b, 2b+1}; each core owns 1024 query points
(q0 = (pid % 2) * 1024). Full cloud features exchanged within each pair via
bf16 AllGather after layers 1-3.

Design vs v1 baseline:
- bf16 compute throughout (PE 1 cyc/row incl. transposes; halved DMA bytes).
- Transposed dma_gather delivers neighbor features feature-major: no PE
  transposes in the edge MLP. Gathers raw x_j (dim D), h1 computed directly
  as wbot^T x_j + u_i via PE (u injected with a selector matmul).
- Top-k via composite packing: u32 = (bf16(score) << 16) | idx. Segment Max8
  (8x256) + 3 merge rounds on 64 candidates; no full-width MaxIndex scans.
- -|xj|^2/2 folded into the dist matmul as two bf16 hi/lo rows.
- max over K neighbors via single tensor_reduce per (qh, m) from PSUM.
- PSUM: dist 2 banks + h1 2 + h2acc <=4 = 8, allows cross-tile pipelining.
"""
import numpy as np
import ml_dtypes

import concourse.bass as bass
import concourse.bacc as bacc
import concourse.mybir as mybir
import concourse.tile as tile
from concourse.bass_utils import run_bass_kernel_spmd

B, P, K = 4, 2048, 20
NQ = 1024
N_CORES = 8
NEG = -3.0e38
F32 = mybir.dt.float32
BF16 = mybir.dt.bfloat16
U32 = mybir.dt.uint32
U16 = mybir.dt.uint16
I16 = mybir.dt.int16
AF = mybir.ActivationFunctionType
ALU = mybir.AluOpType
AX = mybir.AxisListType

#          D  Dpad  DH   DO
LCFG = [(3, 128, 64, 64),
        (64, 128, 128, 128),
        (128, 128, 256, 256),
        (256, 256, 512, 512)]
FC1_CHUNKS = [64, 128, 128, 128, 128, 128, 128, 128]  # 960 rows

_CACHED = {}


def cdiv(a, b):
    return (a + b - 1) // b


def _build():
    nc = bacc.Bacc("TRN2", target_bir_lowering=False, debug=False,
                   num_devices=N_CORES, num_swdge_queues=4)

    # ---------------- DRAM params ----------------
    xTb1_in = nc.declare_dram_parameter("xTb1", [3, P], BF16, isOutput=False)
    xsb1_in = nc.declare_dram_parameter("xsb1", [128, 16 * 128], BF16, isOutput=False)
    nsq1_in = nc.declare_dram_parameter("nsq1", [2, P], BF16, isOutput=False)
    selI_in = nc.declare_dram_parameter("selI", [16, 320], BF16, isOutput=False)
    wp = {}
    for li, (D, DP, DH, DO) in enumerate(LCFG, start=1):
        wp[f"wdiff{li}"] = nc.declare_dram_parameter(f"wdiff{li}", [D, DH], BF16, isOutput=False)
        wp[f"wbot{li}"] = nc.declare_dram_parameter(f"wbot{li}", [DP, DH], BF16, isOutput=False)
        wp[f"ba{li}"] = nc.declare_dram_parameter(f"ba{li}", [2, DH], BF16, isOutput=False)
        wp[f"wb{li}"] = nc.declare_dram_parameter(f"wb{li}", [DH, DO], BF16, isOutput=False)
        wp[f"bb{li}"] = nc.declare_dram_parameter(f"bb{li}", [DO, 1], F32, isOutput=False)
    wp["fw1"] = nc.declare_dram_parameter("fw1", [960, 512], BF16, isOutput=False)
    wp["fb1"] = nc.declare_dram_parameter("fb1", [512, 1], F32, isOutput=False)
    wp["fw2"] = nc.declare_dram_parameter("fw2", [512, 256], BF16, isOutput=False)
    wp["fb2"] = nc.declare_dram_parameter("fb2", [256, 1], F32, isOutput=False)
    wp["fw3"] = nc.declare_dram_parameter("fw3", [256, 1], BF16, isOutput=False)
    wp["fb3"] = nc.declare_dram_parameter("fb3", [1, 1], F32, isOutput=False)
    y_out = nc.declare_dram_parameter("y", [1, NQ], F32, isOutput=True)

    groups = [[2 * b, 2 * b + 1] for b in range(N_CORES // 2)]

    with tile.TileContext(nc) as tc:
        with tc.tile_pool(name="const", bufs=1) as cp, \
             tc.tile_pool(name="glob", bufs=1) as gp, \
             tc.tile_pool(name="dram", bufs=1, space="DRAM") as dram:

            parity = nc.sync.partition_id()
            parity = nc.sync.scalar_reg_alu(ALU.mod, parity, 2)

            selI = cp.tile([16, 320], BF16, name="selI")
            nc.sync.dma_start(selI[:], selI_in[:, :])
            ones2 = cp.tile([2, 128], BF16, name="ones2")
            nc.vector.memset(ones2[:], 1.0)
            onescol = cp.tile([128, 1], BF16, name="onescol")
            nc.vector.memset(onescol[:], 1.0)

            # weights -> SBUF (bf16, direct DMA)
            W = {}
            for li, (D, DP, DH, DO) in enumerate(LCFG, start=1):
                for nm, rows in ((f"wdiff{li}", D), (f"wbot{li}", DP), (f"wb{li}", DH)):
                    tiles = []
                    for c0 in range(0, rows, 128):
                        r = min(128, rows - c0)
                        t = cp.tile([r, wp[nm].shape[1]], BF16, name=f"{nm}_{c0}")
                        nc.sync.dma_start(t[:], wp[nm][c0:c0 + r, :])
                        tiles.append(t)
                    W[nm] = tiles
                t = cp.tile([2, DH], BF16, name=f"ba{li}")
                nc.sync.dma_start(t[:], wp[f"ba{li}"][:, :])
                W[f"ba{li}"] = t
                tiles = []
                for c0 in range(0, DO, 128):
                    r = min(128, DO - c0)
                    t = cp.tile([r, 1], F32, name=f"bb{li}_{c0}")
                    nc.sync.dma_start(t[:], wp[f"bb{li}"][c0:c0 + r, :])
                    tiles.append(t)
                W[f"bb{li}"] = tiles
            fw1_tiles = []
            r0 = 0
            for ci, r in enumerate(FC1_CHUNKS):
                t = cp.tile([r, 512], BF16, name=f"fw1_{ci}")
                nc.sync.dma_start(t[:], wp["fw1"][r0:r0 + r, :])
                fw1_tiles.append(t)
                r0 += r
            fw2_tiles = []
            for c0 in range(0, 512, 128):
                t = cp.tile([128, 256], BF16, name=f"fw2_{c0}")
                nc.sync.dma_start(t[:], wp["fw2"][c0:c0 + 128, :])
                fw2_tiles.append(t)
            fw3_tiles = []
            for c0 in range(0, 256, 128):
                t = cp.tile([128, 1], BF16, name=f"fw3_{c0}")
                nc.sync.dma_start(t[:], wp["fw3"][c0:c0 + 128, :])
                fw3_tiles.append(t)
            fbs = {}
            for nm, w in (("fb1", 512), ("fb2", 256), ("fb3", 1)):
                tiles = []
                for c0 in range(0, w, 128):
                    r = min(128, w - c0)
                    t = cp.tile([r, 1], F32, name=f"{nm}_{c0}")
                    nc.sync.dma_start(t[:], wp[nm][c0:c0 + r, :])
                    tiles.append(t)
                fbs[nm] = tiles

            # persistent double-buffered per-tile structures
            comp = []
            for i in range(2):
                t = gp.tile([128, P], U32, name=f"comp{i}")
                nc.gpsimd.iota(t[:], [[1, P]], base=0, channel_multiplier=0)
                comp.append(t)
            wrapped = []
            for i in range(2):
                t = gp.tile([128, 8 * K], I16, name=f"wrap{i}")
                nc.vector.memset(t[:], 0)
                wrapped.append(t)
            scown = [gp.tile([128, NQ], BF16, name=f"scown{i}") for i in range(8)]
            scoth = [gp.tile([128, NQ], BF16, name=f"scoth{i}") for i in range(2)]
            segtop = [gp.tile([128, 64], F32, name=f"segtop{i}") for i in range(2)]
            top24 = [gp.tile([128, 24], F32, name=f"top24{i}") for i in range(2)]
            idx16 = [gp.tile([128, 24], I16, name=f"idx16{i}") for i in range(2)]
            idx_dram = [dram.tile([128, K], I16, name=f"idxd{i}") for i in range(2)]

            # resident per-layer outputs (feature-major) for the FC head
            xoT = {}
            for li, (_, _, _, DO) in enumerate(LCFG, start=1):
                xoT[li] = [gp.tile([min(128, DO - c0), NQ], BF16,
                                   name=f"xoT{li}_{c0}")
                           for c0 in range(0, DO, 128)]

            ag_in = [dram.tile([do, NQ], BF16, name=f"agin{li}")
                     for li, (_, _, _, do) in enumerate(LCFG[:3], start=1)]
            ag_out = [dram.tile([2 * do, NQ], BF16, name=f"agout{li}")
                      for li, (_, _, _, do) in enumerate(LCFG[:3], start=1)]

            xTb = None     # list of [<=128, P] bf16 feature-major chunks
            nsq2 = None    # [2, P] bf16 hi/lo of -0.5|x|^2

            for li, (D, DP, DH, DO) in enumerate(LCFG, start=1):
                NDC = cdiv(D, 128)     # unpadded contract chunks (dist, u)
                NDCP = DP // 128       # padded contract chunks (gather/h1)
                NHC = cdiv(DH, 128)
                NMC = cdiv(DO, 128)
                h2_bufs = 2 if NMC <= 2 else 1

                with tc.tile_pool(name=f"l{li}", bufs=1) as lp, \
                     tc.tile_pool(name=f"l{li}w", bufs=2) as wkp, \
                     tc.tile_pool(name=f"l{li}ps", bufs=2, space="PSUM") as pdist, \
                     tc.tile_pool(name=f"l{li}h1", bufs=2, space="PSUM") as ph1, \
                     tc.tile_pool(name=f"l{li}h2", bufs=h2_bufs, space="PSUM") as ph2:

                    # ---- layer inputs, own-first index space ----
                    # own queries occupy candidate columns 0..NQ; the twin
                    # core's half occupies NQ..P. xq = own features (local,
                    # pre-AllGather); xoth = twin half (post-AllGather).
                    if li == 1:
                        xq, xoth = [], []
                        t = lp.tile([3, NQ], BF16, name="xq1")
                        nc.sync.dma_start(t[:], xTb1_in[:, 0:NQ])
                        xq.append(t)
                        t = lp.tile([3, NQ], BF16, name="xoth1")
                        nc.sync.dma_start(t[:], xTb1_in[:, NQ:P])
                        xoth.append(t)
                        xsb = lp.tile([128, 16, 128], BF16, name="xsb1")
                        nc.sync.dma_start(
                            xsb[:].rearrange("p r d -> p (r d)"), xsb1_in[:, :])
                        nsq2 = lp.tile([2, P], BF16, name="nsq1")
                        nc.sync.dma_start(nsq2[:], nsq1_in[:, :])
                    else:
                        DPREV = LCFG[li - 2][3]
                        xq = xoT[li - 1]  # own features, already feature-major
                        xsb = lp.tile([128, 16, DP], BF16, name=f"xsb{li}")
                        if DPREV < DP:
                            nc.vector.memset(xsb[:, :, DPREV:DP], 0.0)

                    # pre-AG own-half work: u, xsb own ranks, nsq own quarters
                    usb = []
                    for t in range(8):
                        ups = pdist.tile([128, 512], F32, name="ups", tag="dps")
                        tsl = slice(t * 128, (t + 1) * 128)
                        for ci in range(NDC):
                            nc.tensor.matmul(ups[:, :DH], xq[ci][:, tsl],
                                             W[f"wdiff{li}"][ci][:],
                                             start=(ci == 0), stop=False)
                        nc.tensor.matmul(ups[:, :DH], ones2[:],
                                         W[f"ba{li}"][:], start=False, stop=True)
                        ut = lp.tile([128, DH], BF16, name=f"ust{t}", tag="ust",
                                     bufs=2)
                        nc.scalar.activation(ut[:], ups[:, :DH], AF.Copy)
                        ud = dram.tile([128, DH], BF16, name=f"ud{li}_{t}")
                        nc.sync.dma_start(ud[:, :], ut[:])
                        uq = lp.tile([16, 8, DH], BF16, name=f"usb{t}")
                        nc.sync.dma_start(
                            uq[:], ud[:, :].rearrange("(qh ql) d -> ql qh d", ql=16))
                        usb.append(uq)

                    if li > 1:
                        DPREV = LCFG[li - 2][3]
                        for ci, xt in enumerate(xq):
                            nc.scalar.dma_start_transpose(
                                xsb[:, 0:8, ci * 128:ci * 128 + xt.shape[0]], xt[:])
                        nsq2 = lp.tile([2, P], BF16, name=f"nsq{li}")
                        nsqlo = lp.tile([1, P], BF16, name=f"nsqlo{li}")
                        sqb = lp.tile([128, NQ], BF16, name=f"sqb{li}")

                        def nsq_quarters(src, base):
                            for nb in range(2):
                                nsqps = pdist.tile([128, 512], F32,
                                                   name="nsqps", tag="dps")
                                for ci, xt in enumerate(src):
                                    r = xt.shape[0]
                                    sl = slice(nb * 512, (nb + 1) * 512)
                                    osl = slice(base + nb * 512,
                                                base + (nb + 1) * 512)
                                    nc.vector.tensor_tensor(
                                        sqb[:r, sl], xt[:, sl], xt[:, sl],
                                        op=ALU.mult)
                                    nc.tensor.matmul(
                                        nsqps[0:1, :], onescol[:r, :],
                                        sqb[:r, sl], start=(ci == 0),
                                        stop=(ci == len(src) - 1))
                                nc.scalar.activation(
                                    nsq2[0:1, osl], nsqps[0:1, :],
                                    AF.Copy, scale=-0.5)
                                nc.vector.scalar_tensor_tensor(
                                    nsqlo[0:1, osl], nsqps[0:1, :],
                                    -0.5, nsq2[0:1, osl],
                                    op0=ALU.mult, op1=ALU.subtract)

                        nsq_quarters(xq, 0)
                        nc.sync.dma_start(nsq2[1:2, 0:NQ], nsqlo[0:1, 0:NQ])

                    # phase A: own-half dist for all tiles (pre-AllGather)
                    for t in range(8):
                        tsl = slice(t * 128, (t + 1) * 128)
                        for nb in range(2):
                            dps = pdist.tile([128, 512], F32, name="dpsA", tag="dps")
                            sl = slice(nb * 512, (nb + 1) * 512)
                            for ci in range(NDC):
                                nc.tensor.matmul(dps[:], xq[ci][:, tsl],
                                                 xq[ci][:, sl],
                                                 start=(ci == 0), stop=False)
                            nc.tensor.matmul(dps[:], ones2[:], nsq2[:, sl],
                                             start=False, stop=True)
                            nc.scalar.activation(scown[t][:, sl], dps[:], AF.Copy)

                    if li > 1:
                        DPREV = LCFG[li - 2][3]
                        # post-AG other-half inputs
                        othoff = nc.sync.scalar_reg_alu(ALU.mult, parity, -DPREV)
                        othoff = nc.sync.scalar_reg_alu(ALU.add, othoff, DPREV)
                        xoth = []
                        for c0 in range(0, DPREV, 128):
                            r = min(128, DPREV - c0)
                            rowreg = nc.sync.scalar_reg_alu(ALU.add, othoff, c0)
                            t = lp.tile([r, NQ], BF16, name=f"xoth{li}_{c0}")
                            nc.sync.dma_start(
                                t[:], ag_out[li - 2][bass.ds(rowreg, r), :])
                            xoth.append(t)
                        for ci, xt in enumerate(xoth):
                            nc.scalar.dma_start_transpose(
                                xsb[:, 8:16, ci * 128:ci * 128 + xt.shape[0]], xt[:])
                        nsq_quarters(xoth, NQ)
                        nc.sync.dma_start(nsq2[1:2, NQ:P], nsqlo[0:1, NQ:P])

                    macc = [lp.tile([min(128, DO - c0), NQ], F32,
                                    name=f"macc{li}_{c0}")
                            for c0 in range(0, DO, 128)]

                    # ---- main per-tile loop ----
                    for t in range(8):
                        tb = t % 2
                        tsl = slice(t * 128, (t + 1) * 128)
                        # other-half dist quarters -> score bf16
                        for nb in range(2):
                            dps = pdist.tile([128, 512], F32, name="dps", tag="dps")
                            sl = slice(NQ + nb * 512, NQ + (nb + 1) * 512)
                            rsl = slice(nb * 512, (nb + 1) * 512)
                            for ci in range(NDC):
                                nc.tensor.matmul(dps[:], xq[ci][:, tsl],
                                                 xoth[ci][:, rsl],
                                                 start=(ci == 0), stop=False)
                            nc.tensor.matmul(dps[:], ones2[:], nsq2[:, sl],
                                             start=False, stop=True)
                            nc.scalar.activation(scoth[tb][:, rsl], dps[:], AF.Copy)
                        # composite topk
                        cb = comp[tb]
                        cbu = cb[:].bitcast(U16)[:, 1::2]
                        nc.vector.tensor_copy(cbu[:, 0:NQ],
                                              scown[t][:].bitcast(U16))
                        nc.vector.tensor_copy(cbu[:, NQ:P],
                                              scoth[tb][:].bitcast(U16))
                        compf = cb[:].bitcast(F32)
                        st = segtop[tb]
                        for s in range(8):
                            nc.vector.max(st[:, s * 8:(s + 1) * 8],
                                          compf[:, s * 256:(s + 1) * 256])
                        t24 = top24[tb]
                        for r in range(3):
                            nc.vector.max(t24[:, 8 * r:8 * r + 8], st[:])
                            if r < 2:
                                nc.vector.match_replace(
                                    st[:], t24[:, 8 * r:8 * r + 8], st[:], NEG)
                        nc.vector.tensor_copy(idx16[tb][:],
                                              t24[:].bitcast(I16)[:, 0::2])
                        # wrap indices: dram bounce + 8-block replication
                        nc.sync.dma_start(idx_dram[tb][:, :], idx16[tb][:, 0:K])
                        wsrc = idx_dram[tb][:, :].rearrange(
                            "(qh ql) k -> ql qh k", ql=16)
                        for bb in range(8):
                            nc.sync.dma_start(
                                wrapped[tb][bb * 16:(bb + 1) * 16, :].rearrange(
                                    "ql (qh k) -> ql qh k", k=K), wsrc)
                        # transposed gathers: 4 chunks x 640 edges (2 qh each)
                        vkc = []
                        for g in range(4):
                            vt = wkp.tile([128, NDCP, 640], BF16,
                                          name=f"vk{g}", tag=f"vk{g}")
                            nc.gpsimd.dma_gather(
                                out_ap=vt[:], in_ap=xsb[:].rearrange("p r d -> p (r d)"),
                                idxs_ap=wrapped[tb][:, g * 40:(g + 1) * 40],
                                num_idxs=640, num_idxs_reg=640, elem_size=DP,
                                transpose=True, queue_num=g,
                                sbuf_tokens_per_rank=128,
                                sbuf_free_dim_per_rank=DP * 2,
                                sbuf_free_dim_pad_per_rank=0,
                                sbuf_byte_offset=0)
                            vkc.append(vt)
                        # edge MLP per qh block (320 edges)
                        for qh in range(8):
                            vt = vkc[qh // 2]
                            off = (qh % 2) * 320
                            h1sb = wkp.tile([128, NHC, 320], BF16,
                                            name="h1sb", tag="h1sb")
                            for hc in range(NHC):
                                hr = min(128, DH - hc * 128)
                                h1ps = ph1.tile([128, 320], F32, name="h1ps",
                                                tag="h1ps")
                                for dc in range(NDCP):
                                    nc.tensor.matmul(
                                        h1ps[:hr, :],
                                        W[f"wbot{li}"][dc][:, hc * 128:hc * 128 + hr],
                                        vt[:, dc, off:off + 320],
                                        start=(dc == 0), stop=False)
                                nc.tensor.matmul(
                                    h1ps[:hr, :],
                                    usb[t][:, qh, hc * 128:hc * 128 + hr],
                                    selI[:], start=False, stop=True)
                                nc.scalar.activation(h1sb[:hr, hc, :],
                                                     h1ps[:hr, :], AF.Relu)
                            for m in range(NMC):
                                mr = min(128, DO - m * 128)
                                h2ps = ph2.tile([128, 320], F32, name="h2ps",
                                                tag=f"h2_{m}")
                                for hc in range(NHC):
                                    hr = min(128, DH - hc * 128)
                                    nc.tensor.matmul(
                                        h2ps[:mr, :],
                                        W[f"wb{li}"][hc][:hr, m * 128:m * 128 + mr],
                                        h1sb[:hr, hc, :],
                                        start=(hc == 0), stop=(hc == NHC - 1))
                                nc.vector.tensor_reduce(
                                    macc[m][:mr, t * 128 + qh * 16:
                                            t * 128 + qh * 16 + 16],
                                    h2ps[:mr, :].rearrange("p (k ql) -> p ql k",
                                                           k=K),
                                    axis=AX.X, op=ALU.max)

                    # ---- xo = relu(macc + bb) -> xoT (+ AllGather input) ----
                    for m in range(NMC):
                        mr = min(128, DO - m * 128)
                        nc.scalar.activation(xoT[li][m][:], macc[m][:mr, :],
                                             AF.Relu, bias=W[f"bb{li}"][m][:])
                        if li < 4:
                            nc.sync.dma_start(
                                ag_in[li - 1][m * 128:m * 128 + mr, :],
                                xoT[li][m][:])
                    if li < 4:
                        nc.gpsimd.collective_compute(
                            "AllGather", ALU.bypass, replica_groups=groups,
                            ins=[ag_in[li - 1].opt()],
                            outs=[ag_out[li - 1].opt()])

            # ---------------- FC head ----------------
            with tc.tile_pool(name="fc", bufs=1) as fp, \
                 tc.tile_pool(name="fcps", bufs=2, space="PSUM") as fpp:
                feats = [xoT[1][0], xoT[2][0], xoT[3][0], xoT[3][1],
                         xoT[4][0], xoT[4][1], xoT[4][2], xoT[4][3]]
                h1fc = [fp.tile([128, NQ], BF16, name=f"h1fc{m}") for m in range(4)]
                for m in range(4):
                    for g in range(2):
                        ps = fpp.tile([128, 512], F32, name="fps", tag="fps")
                        gsl = slice(g * 512, (g + 1) * 512)
                        for ci, ft in enumerate(feats):
                            nc.tensor.matmul(ps[:],
                                             fw1_tiles[ci][:, m * 128:(m + 1) * 128],
                                             ft[:, gsl],
                                             start=(ci == 0), stop=(ci == 7))
                        nc.scalar.activation(h1fc[m][:, gsl], ps[:], AF.Relu,
                                             bias=fbs["fb1"][m][:])
                h2fc = [fp.tile([128, NQ], BF16, name=f"h2fc{m}") for m in range(2)]
                for m in range(2):
                    for g in range(2):
                        ps = fpp.tile([128, 512], F32, name="fps2", tag="fps")
                        gsl = slice(g * 512, (g + 1) * 512)
                        for ci in range(4):
                            nc.tensor.matmul(ps[:],
                                             fw2_tiles[ci][:, m * 128:(m + 1) * 128],
                                             h1fc[ci][:, gsl],
                                             start=(ci == 0), stop=(ci == 3))
                        nc.scalar.activation(h2fc[m][:, gsl], ps[:], AF.Relu,
                                             bias=fbs["fb2"][m][:])
                yt = fp.tile([1, NQ], F32, name="yt")
                for g in range(2):
                    ps = fpp.tile([1, 512], F32, name="fps3", tag="fps3")
                    gsl = slice(g * 512, (g + 1) * 512)
                    for ci in range(2):
                        nc.tensor.matmul(ps[:], fw3_tiles[ci][:],
                                         h2fc[ci][:, gsl],
                                         start=(ci == 0), stop=(ci == 1))
                    nc.scalar.activation(yt[:, gsl], ps[:], AF.Sigmoid,
                                         bias=fbs["fb3"][0][:])
                nc.sync.dma_start(y_out[:, :], yt[:])

    nc.compile()
    return nc


def _bf16(a):
    return np.asarray(a, np.float32).astype(ml_dtypes.bfloat16)


def _hilo(row):
    """f32 row -> [2, N] bf16 (hi, residual)."""
    hi = row.astype(ml_dtypes.bfloat16)
    lo = (row - hi.astype(np.float32)).astype(ml_dtypes.bfloat16)
    return np.stack([hi.astype(np.float32), lo.astype(np.float32)]).astype(
        ml_dtypes.bfloat16)


def kernel(**inputs):
    x = np.asarray(inputs["x"], np.float32)          # [8192, 3]
    if "nc" not in _CACHED:
        _CACHED["nc"] = _build()
    nc = _CACHED["nc"]

    selI = np.zeros((16, 320), np.float32)
    for k in range(K):
        for ql in range(16):
            selI[ql, k * 16 + ql] = 1.0

    base = {"selI": selI.astype(ml_dtypes.bfloat16)}
    for li, (D, DP, DH, DO) in enumerate(LCFG, start=1):
        wa = np.asarray(inputs[f"w{li}a"], np.float32)
        wtop, wbot = wa[:D], wa[D:]
        base[f"wdiff{li}"] = _bf16(wtop - wbot)
        wbp = np.zeros((DP, DH), np.float32)
        wbp[:D] = wbot
        base[f"wbot{li}"] = _bf16(wbp)
        base[f"ba{li}"] = _hilo(np.asarray(inputs[f"b{li}a"], np.float32))
        base[f"wb{li}"] = _bf16(inputs[f"w{li}b"])
        base[f"bb{li}"] = np.asarray(inputs[f"b{li}b"], np.float32)[:, None]
    base["fw1"] = _bf16(inputs["fw1"])
    base["fb1"] = np.asarray(inputs["fb1"], np.float32)[:, None]
    base["fw2"] = _bf16(inputs["fw2"])
    base["fb2"] = np.asarray(inputs["fb2"], np.float32)[:, None]
    base["fw3"] = _bf16(inputs["fw3"])
    base["fb3"] = np.asarray(inputs["fb3"], np.float32)[:, None]

    in_maps = []
    for c in range(N_CORES):
        cloud, half = c // 2, c % 2
        xc = x[cloud * P:(cloud + 1) * P]
        # own-first reorder: this core's 1024 query points come first
        xr = np.concatenate([xc[half * NQ:(half + 1) * NQ],
                             xc[(1 - half) * NQ:(2 - half) * NQ]])
        m = dict(base)
        m["xTb1"] = _bf16(xr.T)
        xp = np.zeros((P, 128), np.float32)
        xp[:, :3] = xr
        m["xsb1"] = _bf16(
            xp.reshape(16, 128, 128).transpose(1, 0, 2).reshape(128, 16 * 128))
        m["nsq1"] = _hilo(-0.5 * (xr * xr).sum(1))
        in_maps.append(m)

    res = run_bass_kernel_spmd(nc, in_maps, core_ids=list(range(N_CORES)))
    out = np.empty((B * P, 1), np.float32)
    for c in range(N_CORES):
        cloud, half = c // 2, c % 2
        out[cloud * P + half * NQ: cloud * P + (half + 1) * NQ, 0] = \
            res.results[c]["y"][0]
    return out
